# revision 1
# baseline (speedup 1.0000x reference)
"""Trainium2 Bass kernel for FCGF point-attention + FC head (segment softmax pool).

Pipeline per the nn.Module reference:
  att = relu(bn1(x @ w1.T + b1)) ; att = relu(bn2(att @ w2.T + b2))   [N, K]
  per-segment softmax over points, weighted pool of x -> [B, K, C]
  flatten -> FC -> bn3 -> L2 normalize -> [B, 256]

Distribution: data-parallel over point clouds. 16 segments are paired onto
8 cores (greedy balanced = optimal for this length draw). Each core runs the
conv1x1 stack + segment softmax-pool for its 2 segments, then an AllToAll per
K-half reshards pooled features so each core holds one 4096-wide contraction
chunk of the flattened [16, 32768] pooled matrix for the shared FC. Per-core
FC partials are summed on host with the tiny BN3 + L2-normalize epilogue.

Numerics / speed tricks vs the straightforward version:
- BN1/BN2 folded into conv weights on host.
- conv1 and conv2 run in fp8(e4m3) DoubleRow matmuls: K=2x16 (conv1) and
  K=2x128 (conv2) contracted in a single PE pass at 2x rate.
- softmax max-subtraction skipped (|att| < 4 bounded, exp fits easily).
- the BN2 bias enters as a per-k multiplier e2 = exp(b2') on exp(z). Since
  pooled numerator and denominator Z both scale linearly in the per-k weight,
  e2 cancels in pooled/Z; relu-before-exp becomes w = max(exp(z), 1/e2)
  (exp(relu(z + b2')) * e2^-1... algebra: true w' = e2 * max(exp(z), 1/e2)),
  computed as ONE vector tensor_max against a broadcast 1/e2 row. No
  per-point multiply needed.
- conv1 bias+ReLU moved off the scalar engine (tensor_scalar add+max on
  gpsimd/vector, alternating), so the scalar engine does nothing but Exp.
- pool matmul streams the bf16 weights tile against [x/L | mask] columns,
  accumulating [66, 512] in PSUM across all point blocks; softmax Z and
  pooled sums come from the same matmul.
- PE stream is software-pipelined: conv2 of block b+1 is emitted before the
  pool matmul of block b so the PE never waits on the exp/max chain.
- per-half epilogue (transpose + 1/Z normalize + AllToAll + FC) is emitted a
  few blocks into the NEXT half's compute so collectives and FC overlap the
  main pipeline; only the second half's epilogue is exposed at the tail.
"""

import sys

if "/opt/trn_rl_repo" not in sys.path:
    sys.path.insert(0, "/opt/trn_rl_repo")

import numpy as np
import ml_dtypes

import concourse.bacc as bacc
import concourse.mybir as mybir
import concourse.tile as tile
from concourse.bass_utils import run_bass_kernel_spmd

B, N, C_IN, FC0, K = 16, 32768, 32, 256, 1024
BN_EPS = 1e-5
N_CORES = 8
BLK = 128
BF16 = ml_dtypes.bfloat16
FP8 = ml_dtypes.float8_e4m3

LAST_RESULT = None  # test harness reads exec_time_ns from here
_PROGRAM_CACHE = {}


def _build_program(npair):
    """One SPMD program for all 8 cores; per-core behavior comes from data.

    npair = number of 256-point block pairs per core (point blocks padded
    even so the pool matmul can contract 2x128 points per DoubleRow pass)."""
    nblk = 2 * npair
    P = nblk * BLK
    dt = mybir.dt
    act = mybir.ActivationFunctionType
    alu = mybir.AluOpType
    DR = mybir.MatmulPerfMode.DoubleRow
    nc = bacc.Bacc("TRN2", target_bir_lowering=False, debug=False,
                   num_devices=N_CORES)

    # conv1 point-tile widths (512 except a short remainder)
    c1w = []
    off = 0
    while off < P:
        w = min(512, P - off)
        c1w.append((off, w))
        off += w

    xT_in = nc.dram_tensor("xT8", [16, 2, P], dt.float8e4, kind="ExternalInput")
    # already in [partition, block, col] layout host-side: contiguous DMA
    xab_in = nc.dram_tensor("xab", [128, nblk, 80], dt.float8e4,
                            kind="ExternalInput")
    w1_in = nc.dram_tensor("w1T8", [16, 2, FC0], dt.float8e4, kind="ExternalInput")
    b1_in = nc.dram_tensor("b1c", [128, 2], dt.float32, kind="ExternalInput")
    w2_in = nc.dram_tensor("w28", [128, 2, K], dt.float8e4, kind="ExternalInput")
    e2_in = nc.dram_tensor("e2ir", [128, K], dt.bfloat16, kind="ExternalInput")
    fcw_in = nc.dram_tensor("fcw8", [128, 16, 2, FC0], dt.float8e4,
                            kind="ExternalInput")
    id_in = nc.dram_tensor("ident", [128, 128], dt.float32, kind="ExternalInput")
    idb_in = nc.dram_tensor("identb", [B, B], dt.bfloat16, kind="ExternalInput")
    r_out = nc.dram_tensor("r", [B, FC0], dt.float32, kind="ExternalOutput")

    with tile.TileContext(nc) as tc:
        with (
            tc.tile_pool(name="const", bufs=1) as constp,
            tc.tile_pool(name="es", bufs=4) as es,
            tc.tile_pool(name="ws", bufs=3) as ws,
            tc.tile_pool(name="mis", bufs=3) as mis,
            tc.tile_pool(name="patt", bufs=5, space="PSUM") as patt,
            tc.tile_pool(name="pacc", bufs=2, space="PSUM") as pacc,
            tc.tile_pool(name="pfc", bufs=1, space="PSUM") as pfc,
            tc.tile_pool(name="dram", bufs=1, space="DRAM") as dram,
        ):
            # tiny warm-up collective: absorbs first-call ncfw/communicator
            # setup cost while the input DMA + compute ramp runs
            warm_in = dram.tile([8, 8], dt.float32, tag="warmin")
            warm_out = dram.tile([8, 8], dt.float32, tag="warmout")
            nc.gpsimd.collective_compute(
                "AllToAll", alu.bypass,
                replica_groups=[list(range(N_CORES))],
                ins=[warm_in.opt()], outs=[warm_out.opt()],
            )

            # params + x first (each as ONE batched DMA) so conv1 starts
            # early; e2ir/ident are not needed until later in the pipeline
            w1T8 = constp.tile([16, 2, FC0], dt.float8e4)
            nc.sync.dma_start(w1T8[:], w1_in[:])
            b1c = constp.tile([128, 2], dt.float32)
            nc.sync.dma_start(b1c[:], b1_in[:])
            xT8 = constp.tile([16, 2, P], dt.float8e4)
            nc.gpsimd.dma_start(xT8[:], xT_in[:])
            w28 = constp.tile([128, 2, K], dt.float8e4)
            nc.sync.dma_start(w28[:], w2_in[:])
            e2ir = constp.tile([128, K], dt.bfloat16)
            nc.sync.dma_start(e2ir[:], e2_in[:])
            xabt = constp.tile([128, nblk, 80], dt.float8e4)
            nc.sync.dma_start(xabt[:], xab_in[:])
            ident = constp.tile([128, 128], dt.float32)
            nc.sync.dma_start(ident[:], id_in[:])
            identb = constp.tile([B, B], dt.bfloat16)
            nc.sync.dma_start(identb[:], idb_in[:])
            # bulk FC weights on SWDGE as one batched DMA (gpsimd is
            # otherwise idle until the first collective)
            fcw = constp.tile([128, 16, 2, FC0], dt.float8e4)
            nc.gpsimd.dma_start(fcw[:], fcw_in[:])

            h18 = constp.tile([128, 2, P], dt.float8e4)

            def emit_conv1(t):
                """conv1 tile t: fp8 DoubleRow K=2x16 matmul + bias/relu/cast
                epilogue on the vector engine (scalar only ever runs Exp)."""
                o, w = c1w[t]
                for h in range(2):
                    hp = patt.tile([128, 512], dt.float32, tag="pp")
                    nc.tensor.matmul(hp[:, 0:w],
                                     w1T8[:, :, h * 128:(h + 1) * 128],
                                     xT8[:, :, o:o + w],
                                     start=True, stop=True, perf_mode=DR)
                    nc.vector.tensor_scalar(h18[:, h, o:o + w], hp[:, 0:w],
                                            b1c[:, h:h + 1], 0.0,
                                            alu.add, alu.max)

            # main pipeline over (kq half, point block). Software-pipelined:
            # conv1 tiles are emitted just-in-time two tiles ahead, the pool
            # matmul of pair p is emitted after conv2 of pair p+1's first
            # block so the PE never stalls on the exp/max chain.
            blocks = [(kq, b) for kq in range(2) for b in range(nblk)]
            fc_ps = pfc.tile([B, FC0], dt.float32, tag="fcps")
            pool_accs = []
            pool_q = []
            fcins = {}
            c1_done = 0

            def make_epilogue(kq):
                """Normalize + ship one k-half. Shipments + trigger ride the
                gpsimd queue; the (blocking) fcin loads ride sync so neither
                queue's FIFO can delay the other half's collective."""
                pool_sb = mis.tile([66, 512], dt.float32, tag="poolsb",
                                   name="pool_sb", bufs=2)
                nc.vector.tensor_copy(pool_sb[:], pool_accs[kq][0:66, :])
                o2 = mis.tile([128, 256], dt.bfloat16, tag="o2", name="o2",
                              bufs=2)
                for i in range(4):
                    ptp = patt.tile([128, 66], dt.float32, tag="pp",
                                    name="ptp")
                    nc.tensor.transpose(ptp[:],
                                        pool_sb[:, i * 128:(i + 1) * 128],
                                        ident[0:66, 0:66])
                    for s in range(2):
                        zr = mis.tile([128, 1], dt.float32, tag="zr",
                                      name="zr")
                        nc.vector.reciprocal(
                            zr[:], ptp[:, 33 * s + 32:33 * s + 33])
                        nc.vector.tensor_scalar_mul(
                            o2[:, (i * 2 + s) * 32:(i * 2 + s) * 32 + 32],
                            ptp[:, 33 * s:33 * s + 32], zr[:])
                out2_dram = dram.tile([8, 4096], dt.bfloat16, tag=f"out2d{kq}",
                                      name="out2_dram")
                fcin_dram = dram.tile([8, 4096], dt.bfloat16, tag=f"fcind{kq}",
                                      name="fcin_dram")
                # dst addr for (p, i, s, c): (i*2 + p//64)*4096 + s*2048
                #                            + (p%64)*32 + c
                dst = out2_dram[:].rearrange(
                    "(i two) (s p c) -> two s p i c",
                    two=2, s=2, p=64, c=32)
                src4 = o2[:].rearrange("p (i s c) -> s p i c", i=4, s=2,
                                       c=32)
                for ph in range(2):
                    for s in range(2):
                        eng = nc.sync if s == 0 else nc.gpsimd
                        eng.dma_start(
                            dst[ph, s],
                            src4[s, ph * 64:(ph + 1) * 64])
                nc.gpsimd.collective_compute(
                    "AllToAll", alu.bypass,
                    replica_groups=[list(range(N_CORES))],
                    ins=[out2_dram.opt()], outs=[fcin_dram.opt()],
                )
                fcins[kq] = fcin_dram

            def make_fc(kq):
                fcin = fcins[kq]
                for j in range(8):
                    lhs = mis.tile([128, 2, B], dt.float8e4, tag="fclhs",
                                   name="lhs")
                    for i in range(2):
                        ftp = patt.tile([128, B], dt.bfloat16, tag="pp",
                                        name="ftp")
                        nc.tensor.transpose(
                            ftp[:],
                            fcin[:, (2 * j + i) * 128:(2 * j + i + 1) * 128],
                            identb[0:B, 0:B])
                        nc.vector.tensor_copy(lhs[:, i, :], ftp[:])
                    nc.tensor.matmul(fc_ps[:], lhs[:],
                                     fcw[:, kq * 8 + j, :, :],
                                     start=(kq == 0 and j == 0),
                                     stop=(kq == 1 and j == 7),
                                     perf_mode=DR, skip_group_check=True)

            def emit_pool():
                kq, pr, wd = pool_q.pop(0)
                nc.tensor.matmul(pool_accs[kq][:],
                                 xabt[:, 2 * pr:2 * pr + 2, :], wd[:],
                                 start=(pr == 0), stop=(pr == npair - 1),
                                 perf_mode=DR, skip_group_check=True)

            wd = None
            for kq, b in blocks:
                if kq == 0 and b % 4 == 0 and c1_done < len(c1w):
                    # JIT conv1: stay two 512-tiles (8 blocks) ahead
                    want = min(len(c1w), b // 4 + 3 if b else 3)
                    while c1_done < want:
                        emit_conv1(c1_done)
                        c1_done += 1
                if b == 0:
                    pa = pacc.tile([80, 512], dt.float32, tag="pacc",
                                   name=f"pacc{kq}")
                    pool_accs.append(pa)
                ksl = slice(kq * 512, (kq + 1) * 512)
                ap_ = patt.tile([128, 512], dt.float32, tag="pp")
                nc.tensor.matmul(ap_[:], h18[:, :, b * BLK:(b + 1) * BLK],
                                 w28[:, :, ksl], start=True, stop=True,
                                 perf_mode=DR)
                e_sb = es.tile([128, 512], dt.bfloat16, tag="e")
                if b % 2 == 0:
                    wd = ws.tile([128, 2, 512], dt.float8e4, tag="w",
                                 name="wd")
                nc.scalar.activation(e_sb[:], ap_[:], act.Exp)
                nc.vector.tensor_max(wd[:, b % 2, :], e_sb[:], e2ir[:, ksl])
                if b % 2 == 1:
                    pool_q.append((kq, b // 2, wd))
                if len(pool_q) >= 2:
                    emit_pool()
                # kq0's normalize + shipment + collective overlap kq1's
                # compute; its FC waits for the tail (fcin gated on the
                # serialized cc stream - emitting it earlier would
                # head-of-line block the PE)
                if kq == 1 and b == 2:
                    make_epilogue(0)
            while pool_q:
                emit_pool()
            make_epilogue(1)
            for kq in range(2):
                fcin = mis.tile([B, 2048], dt.bfloat16, tag=f"fcin{kq}",
                                name="fcin")
                nc.gpsimd.dma_start(
                    fcin[:],
                    fcins[kq][:].rearrange("a (s x) -> (a s) x", s=2))
                fcins[kq] = fcin
            make_fc(0)
            make_fc(1)

            r_sb = mis.tile([B, FC0], dt.float32, tag="rsb")
            nc.vector.tensor_copy(r_sb[:], fc_ps[:])
            nc.sync.dma_start(r_out[:], r_sb[:])

    nc.compile()
    return nc


def _segment_runs(length):
    """Contiguous [start, end) row-run per segment, mirroring
    jnp.repeat(arange(B), length, total_repeat_length=N)."""
    length = np.asarray(length, np.int64)
    seg = np.repeat(np.arange(B), np.maximum(length, 0))
    if len(seg) >= N:
        seg = seg[:N]
    else:
        seg = np.pad(seg, (0, N - len(seg)), constant_values=B - 1)
    runs = []
    for b in range(B):
        idx = np.nonzero(seg == b)[0]
        if len(idx):
            runs.append((int(idx[0]), int(idx[-1]) + 1))
        else:
            runs.append((0, 0))
    return runs


def _pair_segments(runs):
    """Greedy balanced pairing: largest with smallest."""
    sizes = np.array([e - s for s, e in runs])
    order = list(np.argsort(-sizes))
    pairs = [(int(order[i]), int(order[B - 1 - i])) for i in range(B // 2)]
    return pairs


def kernel(**inputs):
    global LAST_RESULT
    f32 = np.float32
    x = np.asarray(inputs["x"], f32)
    length = np.asarray(inputs["length"])
    w1 = np.asarray(inputs["w1"], f32); b1 = np.asarray(inputs["b1"], f32)
    g1 = np.asarray(inputs["g1"], f32); be1 = np.asarray(inputs["be1"], f32)
    m1 = np.asarray(inputs["m1"], f32); v1 = np.asarray(inputs["v1"], f32)
    w2 = np.asarray(inputs["w2"], f32); b2 = np.asarray(inputs["b2"], f32)
    g2 = np.asarray(inputs["g2"], f32); be2 = np.asarray(inputs["be2"], f32)
    m2 = np.asarray(inputs["m2"], f32); v2 = np.asarray(inputs["v2"], f32)
    fcw = np.asarray(inputs["fcw"], f32); fcb = np.asarray(inputs["fcb"], f32)
    g3 = np.asarray(inputs["g3"], f32); be3 = np.asarray(inputs["be3"], f32)
    m3 = np.asarray(inputs["m3"], f32); v3 = np.asarray(inputs["v3"], f32)

    # fold BN1/BN2 into the conv weights
    a1 = g1 / np.sqrt(v1 + BN_EPS)
    w1p = (a1[:, None] * w1).astype(f32)
    b1p = (a1 * (b1 - m1) + be1).astype(f32)
    a2 = g2 / np.sqrt(v2 + BN_EPS)
    w2p = (a2[:, None] * w2).astype(f32)
    b2p = (a2 * (b2 - m2) + be2).astype(f32)
    e2inv = np.exp(-b2p).astype(f32)

    runs = _segment_runs(length)
    pairs = _pair_segments(runs)
    lenf = np.asarray(length, f32)
    max_pair = max(
        (runs[a][1] - runs[a][0]) + (runs[b][1] - runs[b][0]) for a, b in pairs
    )
    npair = max(1, -(-int(max_pair) // (2 * BLK)))
    nblk = 2 * npair
    P = nblk * BLK

    # shared parameter tensors in DoubleRow layouts
    w1T8 = np.ascontiguousarray(
        w1p.T.reshape(2, 16, FC0).transpose(1, 0, 2)).astype(FP8)
    b1c = b1p.reshape(2, 128).T.astype(f32).copy()          # [128, 2]
    w28 = np.ascontiguousarray(
        w2p.T.reshape(2, 128, K).transpose(1, 0, 2)).astype(FP8)
    e2ir = np.broadcast_to(e2inv, (128, K)).astype(BF16).copy()
    # x64 scale keeps the small FC weights in e4m3's normal range; undone
    # on the host after the kernel returns
    fcwT = (64.0 * fcw.T).astype(f32)                       # [32768, 256]
    ident = np.eye(128, dtype=f32)

    in_maps = []
    for c, (sa, sb) in enumerate(pairs):
        (a0, a1e), (b0, b1e) = runs[sa], runs[sb]
        nA, nB = a1e - a0, b1e - b0
        xc = np.zeros((P, C_IN), f32)
        xc[:nA] = x[a0:a1e]
        xc[nA:nA + nB] = x[b0:b1e]
        # raw x in the pool columns (the /length moves to the host epilogue)
        # so the fp8 cast stays in e4m3's normal range
        xab = np.zeros((P, 80), f32)
        if nA:
            xab[:nA, 0:32] = x[a0:a1e]
            xab[:nA, 32] = 1.0
        if nB:
            xab[nA:nA + nB, 33:65] = x[b0:b1e]
            xab[nA:nA + nB, 65] = 1.0
        # FC contraction chunk for core c: k in [kq*512 + c*64, +64) per half,
        # paired into the DoubleRow [p, chunk-pair, half, f] layout
        fcw_c = np.vstack([fcwT[c * 2048:(c + 1) * 2048],
                           fcwT[16384 + c * 2048:16384 + (c + 1) * 2048]])
        fcw8 = np.ascontiguousarray(
            fcw_c.reshape(16, 2, 128, FC0).transpose(2, 0, 1, 3)).astype(FP8)
        xT8 = np.ascontiguousarray(
            xc.T.reshape(2, 16, P).transpose(1, 0, 2)).astype(FP8)
        in_maps.append({
            "xT8": xT8,
            "xab": np.ascontiguousarray(
                xab.reshape(nblk, 128, 80).transpose(1, 0, 2)).astype(FP8),
            "w1T8": w1T8, "b1c": b1c, "w28": w28, "e2ir": e2ir,
            "fcw8": fcw8,
            "ident": ident, "identb": np.eye(B, dtype=BF16),
        })

    if npair not in _PROGRAM_CACHE:
        _PROGRAM_CACHE[npair] = _build_program(npair)
    nc = _PROGRAM_CACHE[npair]

    res = run_bass_kernel_spmd(nc, in_maps, list(range(N_CORES)))
    LAST_RESULT = res

    r = np.zeros((B, FC0), f32)
    for c in range(N_CORES):
        r += res.results[c]["r"]
    r *= 1.0 / 64.0  # undo the fcw fp8 scale
    # pooled was accumulated from raw x; divide by segment length here
    linv = np.empty((B, 1), f32)
    for c, (sa, sb) in enumerate(pairs):
        linv[2 * c] = 1.0 / max(lenf[sa], 1e-30)
        linv[2 * c + 1] = 1.0 / max(lenf[sb], 1e-30)
    r *= linv
    r += fcb
    a3 = g3 / np.sqrt(v3 + BN_EPS)
    r = (r - m3) * a3 + be3
    r = r / np.maximum(np.linalg.norm(r, axis=1, keepdims=True), 1e-12)

    # rows are in (core, pair-slot) order; map back to segment order
    out = np.empty((B, FC0), f32)
    for c, (sa, sb) in enumerate(pairs):
        out[sa] = r[2 * c]
        out[sb] = r[2 * c + 1]
    return out.astype(np.float32)

